# revision 2
# baseline (speedup 1.0000x reference)
"""BitNet ternary linear (nn_BitNetLinear4Bit) Trainium2 Bass kernel.

out = x @ (alpha * clip(round(w/alpha), -1, 1))^T + bias
  x: [2, 2048, 4096] f32, w: [11008, 4096] f32, alpha: [1] f32, bias: [11008] f32
  -> out: [2, 2048, 11008] f32

Sharding: column-parallel over 8 cores. Each core gets the full x
(replicated) and a 1376-row slice of w / bias; it produces a
[4096, 1376] slice of the output which the host concatenates.

Per-core algorithm — mixed-precision K split to cut PE time:
  k < KBF (=2048): bf16 path. x cast to bf16, XBAR-transposed;
    w ternarized to bf16 and PE-transposed. 16 normal bf16 matmuls.
  k >= KBF: fp8e4 path at 2x rate via DoubleRow. x cast to e4m3 and
    XBAR-transposed as uint16 byte-pairs, then DVE de-interleaved into
    even/odd slot blocks; w ternarized to e4m3 (exact: {-1,0,1}) and
    transposed the same way. 8 DoubleRow matmuls of 256-k each.
  Both accumulate into the same PSUM bank; evict ACT copy*alpha then
  GpSimd +bias; DMA out. Expected rel err ~1.66e-2 (fp8 quantization
  of half of x), verified against an exact CPU model.

alpha is read on the host and baked into the program as an immediate;
the compiled program is cached keyed on alpha.
"""

import numpy as np

B, S, DIN, DOUT = 2, 2048, 4096, 11008
NCORES = 8
DOUT_SH = DOUT // NCORES  # 1376
TOK = B * S  # 4096
P = 128
KBF = 2048  # bf16 k range; the rest is fp8 DoubleRow
K8 = DIN - KBF
KO_BF = KBF // P  # 16
NB8 = K8 // (2 * P)  # 8 DoubleRow blocks of 256 k


def _build(alpha_f, debug=False):
    import concourse.mybir as mybir
    from concourse import bacc
    from concourse.tile import TileContext

    f32 = mybir.dt.float32
    bf16 = mybir.dt.bfloat16
    f8 = mybir.dt.float8e4
    u16 = mybir.dt.uint16
    Alu = mybir.AluOpType
    Act = mybir.ActivationFunctionType
    DR = mybir.MatmulPerfMode.DoubleRow

    M_SUBS = TOK // P  # 32
    W_CHUNKS = (DOUT_SH + P - 1) // P  # 11 (last chunk 96 rows, zero-padded)
    QCOL = 1024
    NQ = DIN // QCOL  # 4 q-blocks; q=0,1 bf16, q=2,3 fp8
    # output groups: one psum bank each, up to 4 chunks (<=512 cols incl pad)
    GROUPS = []  # (first chunk, n chunks, dout start, real width)
    c = 0
    while c < W_CHUNKS:
        cc = min(4, W_CHUNKS - c)
        width = min(DOUT_SH, (c + cc) * P) - c * P
        GROUPS.append((c, cc, c * P, width))
        c += cc

    a2 = float(alpha_f) * 0.5

    nc = bacc.Bacc(None, target_bir_lowering=False, debug=debug)
    x_d = nc.dram_tensor("x", [TOK, DIN], f32, kind="ExternalInput")
    w_d = nc.dram_tensor("w", [DOUT_SH, DIN], f32, kind="ExternalInput")
    nc.dram_tensor("alpha", [1], f32, kind="ExternalInput")
    b_d = nc.dram_tensor("bias", [DOUT_SH], f32, kind="ExternalInput")
    o_d = nc.dram_tensor("out", [TOK, DOUT_SH], f32, kind="ExternalOutput")

    from concourse.masks import make_identity

    with TileContext(nc) as tc:
        with (
            tc.tile_pool(name="const", bufs=1) as const,
            tc.tile_pool(name="wres", bufs=1) as wres,
            tc.tile_pool(name="ptp", bufs=2, space="PSUM") as ptp,
        ):
            ident = const.tile([P, P], bf16)
            make_identity(nc, ident)
            bias_sb = const.tile([P, DOUT_SH], f32)
            nc.sync.dma_start(
                bias_sb[:],
                b_d[:].rearrange("(a n) -> a n", a=1).to_broadcast((P, DOUT_SH)),
            )

            # resident transposed ternary weights per output group:
            #   bf16: wtg_bf[g][p, i, ko, j] = t[(c0+i)*128 + j, ko*128 + p]
            #   fp8:  wt8[g][p, b, sl, i*128+j] = t[(c0+i)*128 + j,
            #                    KBF + 256*b + 2*p + sl]   (even/odd pairing)
            wtg_bf = [
                wres.tile([P, cc, KO_BF, P], bf16, name=f"wtgbf_{g}")
                for g, (_, cc, _, _) in enumerate(GROUPS)
            ]
            wt8 = [
                wres.tile([P, NB8, 2, cc * P], f8, name=f"wt8_{g}")
                for g, (_, cc, _, _) in enumerate(GROUPS)
            ]

            # ---- Phase W: quantize + transpose w shard ----
            with tc.tile_pool(name="wq", bufs=4) as wq:
                for g, (c0, cc, n0, width) in enumerate(GROUPS):
                    for i in range(cc):
                        c = c0 + i
                        rc = min(P, DOUT_SH - c * P)  # 128 or 96 (last)
                        for q in range(NQ):
                            wrow = wq.tile([P, QCOL], f32, tag="wrow")
                            if rc < P:
                                nc.gpsimd.memset(wrow[:], 0.0)
                            nc.sync.dma_start(
                                wrow[:rc, :],
                                w_d[c * P : c * P + rc, q * QCOL : (q + 1) * QCOL],
                            )
                            # t = (w >= a/2) - (w <= -a/2) in {-1,0,1}
                            le = wq.tile([P, QCOL], bf16, tag="le")
                            nc.vector.tensor_scalar(
                                le[:], wrow[:], -a2, None, Alu.is_le
                            )
                            if q < 2:
                                # bf16 path: PE-transpose 128x128 blocks
                                tq = wq.tile([P, QCOL], bf16, tag="tq")
                                nc.vector.scalar_tensor_tensor(
                                    tq[:], wrow[:], a2, le[:], Alu.is_ge, Alu.subtract
                                )
                                for bb in range(QCOL // P):
                                    pt = ptp.tile([P, P], bf16, tag="pt")
                                    nc.tensor.transpose(
                                        pt[:], tq[:, bb * P : (bb + 1) * P], ident[:]
                                    )
                                    nc.any.tensor_copy(
                                        wtg_bf[g][:, i, q * (QCOL // P) + bb, :],
                                        pt[:],
                                    )
                            else:
                                # fp8 path: u16 XBAR transpose + de-interleave
                                tq8 = wq.tile([P, QCOL], f8, tag="tq8")
                                nc.vector.scalar_tensor_tensor(
                                    tq8[:], wrow[:], a2, le[:], Alu.is_ge, Alu.subtract
                                )
                                tt16 = wq.tile([P, QCOL // 256, P], u16, tag="tt16")
                                nc.sync.dma_start_transpose(
                                    tt16[:], tq8[:].bitcast(u16)
                                )
                                ttf8 = tt16[:].bitcast(f8).rearrange(
                                    "p b (d two) -> p b d two", two=2
                                )
                                for bb in range(QCOL // 256):
                                    b = (q - 2) * (QCOL // 256) + bb
                                    for sl in range(2):
                                        nc.vector.tensor_copy(
                                            wt8[g][:, b, sl, i * P : (i + 1) * P],
                                            ttf8[:, bb, :, sl],
                                        )

            # ---- Phase MM ----
            with (
                tc.tile_pool(name="xp", bufs=2) as xp,
                tc.tile_pool(name="xcp", bufs=2) as xcp,
                tc.tile_pool(name="xtp", bufs=3) as xtp,
                tc.tile_pool(name="op", bufs=4) as op,
                tc.tile_pool(name="pso", bufs=6, space="PSUM") as pso,
            ):
                for ms in range(M_SUBS):
                    xrow = xp.tile([P, DIN], f32, tag="xrow")
                    for h in range(4):
                        hw = DIN // 4
                        nc.sync.dma_start(
                            xrow[:, h * hw : (h + 1) * hw],
                            x_d[ms * P : (ms + 1) * P, h * hw : (h + 1) * hw],
                        )
                    xbf = xcp.tile([P, KBF], bf16, tag="xbf")
                    nc.vector.tensor_copy(xbf[:], xrow[:, :KBF])
                    x8 = xcp.tile([P, K8], f8, tag="x8")
                    nc.vector.tensor_copy(x8[:], xrow[:, KBF:])

                    xt_bf = xtp.tile([P, KO_BF, P], bf16, tag="xtbf")
                    nc.sync.dma_start_transpose(xt_bf[:], xbf[:])
                    xt16 = xtp.tile([P, NB8, P], u16, tag="xt16")
                    nc.sync.dma_start_transpose(xt16[:], x8[:].bitcast(u16))
                    xtf8 = xt16[:].bitcast(f8).rearrange(
                        "p b (t two) -> p b t two", two=2
                    )
                    xt8 = xtp.tile([P, NB8, 2, P], f8, tag="xt8")
                    for b in range(NB8):
                        for sl in range(2):
                            nc.vector.tensor_copy(
                                xt8[:, b, sl, :], xtf8[:, b, :, sl]
                            )

                    for g, (c0, cc, n0, width) in enumerate(GROUPS):
                        po = pso.tile([P, 512], f32, tag="po", name=f"po_{ms}_{g}")
                        pw = cc * P  # padded width (>= real width)
                        for ko in range(KO_BF):
                            nc.tensor.matmul(
                                po[:, :pw],
                                xt_bf[:, ko, :],
                                wtg_bf[g][:, :, ko, :],
                                start=(ko == 0),
                                stop=False,
                            )
                        for b in range(NB8):
                            nc.tensor.matmul(
                                po[:, :pw],
                                xt8[:, b, :, :],
                                wt8[g][:, b, :, :],
                                start=False,
                                stop=(b == NB8 - 1),
                                perf_mode=DR,
                            )
                        # out = psum * alpha (ACT), then += bias (GpSimd)
                        osb = op.tile([P, 512], f32, tag="osb", name=f"osb_{ms}_{g}")
                        nc.scalar.activation(
                            osb[:, :width],
                            po[:, :width],
                            Act.Copy,
                            scale=float(alpha_f),
                        )
                        nc.gpsimd.tensor_add(
                            osb[:, :width],
                            osb[:, :width],
                            bias_sb[:, n0 : n0 + width],
                        )
                        nc.sync.dma_start(
                            o_d[ms * P : (ms + 1) * P, n0 : n0 + width],
                            osb[:, :width],
                        )

    nc.compile()
    return nc


_CACHE = {}


def _get_nc(alpha_f):
    key = float(alpha_f)
    if key not in _CACHE:
        _CACHE[key] = _build(key)
    return _CACHE[key]


def kernel(x, w, alpha, bias):
    from concourse.bass_utils import run_bass_kernel_spmd

    alpha2 = np.ascontiguousarray(np.asarray(alpha, dtype=np.float32).reshape(1))
    nc = _get_nc(alpha2[0])
    x2 = np.ascontiguousarray(np.asarray(x, dtype=np.float32).reshape(TOK, DIN))
    in_maps = []
    for c in range(NCORES):
        in_maps.append(
            {
                "x": x2,
                "w": np.ascontiguousarray(w[c * DOUT_SH : (c + 1) * DOUT_SH]),
                "alpha": alpha2,
                "bias": np.ascontiguousarray(bias[c * DOUT_SH : (c + 1) * DOUT_SH]),
            }
        )
    res = run_bass_kernel_spmd(nc, in_maps, core_ids=list(range(NCORES)))
    outs = [res.results[c]["out"] for c in range(NCORES)]
    out = np.concatenate(outs, axis=1).reshape(B, S, DOUT)
    return np.ascontiguousarray(out.astype(np.float32))


# revision 5
# speedup vs baseline: 1.0448x; 1.0448x over previous
"""BitNet ternary linear (nn_BitNetLinear4Bit) Trainium2 Bass kernel.

out = x @ (alpha * clip(round(w/alpha), -1, 1))^T + bias
  x: [2, 2048, 4096] f32, w: [11008, 4096] f32, alpha: [1] f32, bias: [11008] f32
  -> out: [2, 2048, 11008] f32

Sharding: column-parallel over 8 cores. Each core gets the full x
(replicated) and a 1376-row slice of w / bias; it produces a
[4096, 1376] slice of the output which the host concatenates.

Per-core algorithm — mixed-precision K split to cut PE time:
  k < KBF (=2048): bf16 path. x cast to bf16, XBAR-transposed;
    w ternarized to bf16 and PE-transposed. 16 normal bf16 matmuls.
  k >= KBF: fp8e4 path at 2x rate via DoubleRow. x cast to e4m3,
    XBAR-transposed as uint16 byte-pairs, then DVE de-interleaved
    into even/odd slot blocks (the ISA requires a block layout for
    the stationary operand); w ternarized to e4m3 (exact {-1,0,1})
    and XBAR-transposed the same way. The moving (weight) operand
    reads the byte-interleaved pairs directly with a stride-2 AP if
    W_STRIDED_MOVING, else it is de-interleaved too. 8 DoubleRow
    matmuls of 256-k each.
  Both paths accumulate into the same PSUM bank; evict ACT
  copy*alpha then GpSimd +bias; DMA out on the ACT HWDGE ring.
  The per-ms prep (loads/casts/XBARs) is emitted one iteration ahead
  of the compute stage so the strict-FIFO engine queues pipeline.
  Expected rel err ~1.66e-2 (fp8 quantization of half of x),
  verified against an exact CPU model.

alpha is read on the host and baked into the program as an immediate;
the compiled program is cached keyed on alpha.
"""

import numpy as np

B, S, DIN, DOUT = 2, 2048, 4096, 11008
NCORES = 8
DOUT_SH = DOUT // NCORES  # 1376
TOK = B * S  # 4096
P = 128
KBF = 2048  # bf16 k range; the rest is fp8 DoubleRow
K8 = DIN - KBF
KO_BF = KBF // P  # 16
NB8 = K8 // (2 * P)  # 8 DoubleRow blocks of 256 k

W_STRIDED_MOVING = True  # feed DR moving operand via stride-2 byte AP


def _build(alpha_f, debug=False):
    import concourse.mybir as mybir
    from concourse import bacc
    from concourse.tile import TileContext

    f32 = mybir.dt.float32
    bf16 = mybir.dt.bfloat16
    f8 = mybir.dt.float8e4
    u16 = mybir.dt.uint16
    Alu = mybir.AluOpType
    Act = mybir.ActivationFunctionType
    DR = mybir.MatmulPerfMode.DoubleRow

    M_SUBS = TOK // P  # 32
    W_CHUNKS = (DOUT_SH + P - 1) // P  # 11 (last chunk 96 rows, zero-padded)
    QCOL = 1024
    NQ = DIN // QCOL  # 4 q-blocks; q=0,1 bf16, q=2,3 fp8
    GROUPS = []  # (first chunk, n chunks, dout start, real width)
    c = 0
    while c < W_CHUNKS:
        cc = min(4, W_CHUNKS - c)
        width = min(DOUT_SH, (c + cc) * P) - c * P
        GROUPS.append((c, cc, c * P, width))
        c += cc

    a2 = float(alpha_f) * 0.5

    nc = bacc.Bacc(None, target_bir_lowering=False, debug=debug)
    x_d = nc.dram_tensor("x", [TOK, DIN], f32, kind="ExternalInput")
    w_d = nc.dram_tensor("w", [DOUT_SH, DIN], f32, kind="ExternalInput")
    nc.dram_tensor("alpha", [1], f32, kind="ExternalInput")
    b_d = nc.dram_tensor("bias", [DOUT_SH], f32, kind="ExternalInput")
    o_d = nc.dram_tensor("out", [TOK, DOUT_SH], f32, kind="ExternalOutput")

    from concourse.masks import make_identity

    with TileContext(nc) as tc:
        with (
            tc.tile_pool(name="const", bufs=1) as const,
            tc.tile_pool(name="wres", bufs=1) as wres,
            tc.tile_pool(name="ptp", bufs=2, space="PSUM") as ptp,
        ):
            ident = const.tile([P, P], bf16)
            make_identity(nc, ident)
            bias_sb = const.tile([P, DOUT_SH], f32)
            nc.sync.dma_start(
                bias_sb[:],
                b_d[:].rearrange("(a n) -> a n", a=1).to_broadcast((P, DOUT_SH)),
            )

            # resident transposed ternary weights per output group:
            #   bf16: wtg_bf[g][p, i, ko, j] = t[(c0+i)*128 + j, ko*128 + p]
            #   fp8 u16 pairs: wt16[g][p, b, i, j] = bytes
            #     (t[(c0+i)*128+j, KBF+256b+2p], t[..., KBF+256b+2p+1])
            wtg_bf = [
                wres.tile([P, cc, KO_BF, P], bf16, name=f"wtgbf_{g}")
                for g, (_, cc, _, _) in enumerate(GROUPS)
            ]
            wt16 = [
                wres.tile([P, NB8, cc, P], u16, name=f"wt16_{g}")
                for g, (_, cc, _, _) in enumerate(GROUPS)
            ]
            if not W_STRIDED_MOVING:
                wt8 = [
                    wres.tile([P, NB8, 2, cc * P], f8, name=f"wt8_{g}")
                    for g, (_, cc, _, _) in enumerate(GROUPS)
                ]

            # ---- Phase W: quantize + transpose w shard ----
            with tc.tile_pool(name="wq", bufs=4) as wq:
                for g, (c0, cc, n0, width) in enumerate(GROUPS):
                    for i in range(cc):
                        c = c0 + i
                        rc = min(P, DOUT_SH - c * P)  # 128 or 96 (last)
                        for q in range(NQ):
                            wrow = wq.tile([P, QCOL], f32, tag="wrow")
                            if rc < P:
                                nc.gpsimd.memset(wrow[:], 0.0)
                            nc.sync.dma_start(
                                wrow[:rc, :],
                                w_d[c * P : c * P + rc, q * QCOL : (q + 1) * QCOL],
                            )
                            # t = (w >= a/2) - (w <= -a/2) in {-1,0,1}
                            le = wq.tile([P, QCOL], bf16, tag="le")
                            nc.vector.tensor_scalar(
                                le[:], wrow[:], -a2, None, Alu.is_le
                            )
                            if q < 2:
                                # bf16 path: PE-transpose 128x128 blocks
                                tq = wq.tile([P, QCOL], bf16, tag="tq")
                                nc.vector.scalar_tensor_tensor(
                                    tq[:], wrow[:], a2, le[:], Alu.is_ge, Alu.subtract
                                )
                                for bb in range(QCOL // P):
                                    pt = ptp.tile([P, P], bf16, tag="pt")
                                    nc.tensor.transpose(
                                        pt[:], tq[:, bb * P : (bb + 1) * P], ident[:]
                                    )
                                    nc.any.tensor_copy(
                                        wtg_bf[g][:, i, q * (QCOL // P) + bb, :],
                                        pt[:],
                                    )
                            else:
                                # fp8 path: u16 XBAR transpose into pair layout
                                tq8 = wq.tile([P, QCOL], f8, tag="tq8")
                                nc.vector.scalar_tensor_tensor(
                                    tq8[:], wrow[:], a2, le[:], Alu.is_ge, Alu.subtract
                                )
                                b0 = (q - 2) * (QCOL // 256)
                                nc.sync.dma_start_transpose(
                                    wt16[g][:, b0 : b0 + QCOL // 256, i, :],
                                    tq8[:].bitcast(u16),
                                )
                if not W_STRIDED_MOVING:
                    # de-interleave pairs into slot-block layout
                    for g, (c0, cc, n0, width) in enumerate(GROUPS):
                        wv = wt16[g][:].bitcast(f8).rearrange(
                            "p b c (d two) -> p b two (c d)", two=2
                        )
                        for sl in range(2):
                            nc.vector.tensor_copy(
                                wt8[g][:, :, sl, :], wv[:, :, sl, :]
                            )

            # ---- Phase MM: prep emitted one ms ahead of compute ----
            with (
                tc.tile_pool(name="xp", bufs=3) as xp,
                tc.tile_pool(name="xcp", bufs=3) as xcp,
                tc.tile_pool(name="xtp", bufs=3) as xtp,
                tc.tile_pool(name="op", bufs=4) as op,
                tc.tile_pool(name="pso", bufs=6, space="PSUM") as pso,
            ):
                xt_tiles = {}

                def emit_prep(ms):
                    xrow = xp.tile([P, DIN], f32, tag="xrow")
                    for h in range(4):
                        hw = DIN // 4
                        nc.sync.dma_start(
                            xrow[:, h * hw : (h + 1) * hw],
                            x_d[ms * P : (ms + 1) * P, h * hw : (h + 1) * hw],
                        )
                    xbf = xcp.tile([P, KBF], bf16, tag="xbf")
                    nc.vector.tensor_copy(xbf[:], xrow[:, :KBF])
                    x8 = xcp.tile([P, K8], f8, tag="x8")
                    nc.vector.tensor_copy(x8[:], xrow[:, KBF:])

                    xt_bf = xtp.tile([P, KO_BF, P], bf16, tag="xtbf")
                    nc.sync.dma_start_transpose(xt_bf[:], xbf[:])
                    xt16 = xtp.tile([P, NB8, P], u16, tag="xt16")
                    nc.sync.dma_start_transpose(xt16[:], x8[:].bitcast(u16))
                    # de-interleave to slot-block layout for the stationary
                    # operand (ISA requires block layout): 2 big copies
                    xtf8 = xt16[:].bitcast(f8).rearrange(
                        "p b (t two) -> p b two t", two=2
                    )
                    xt8 = xtp.tile([P, NB8, 2, P], f8, tag="xt8")
                    for sl in range(2):
                        nc.vector.tensor_copy(xt8[:, :, sl, :], xtf8[:, :, sl, :])
                    xt_tiles[ms] = (xt_bf, xt8)

                def emit_compute(ms):
                    xt_bf, xt8 = xt_tiles.pop(ms)
                    for g, (c0, cc, n0, width) in enumerate(GROUPS):
                        po = pso.tile([P, 512], f32, tag="po", name=f"po_{ms}_{g}")
                        pw = cc * P  # padded width (>= real width)
                        if W_STRIDED_MOVING:
                            w8v = wt16[g][:].bitcast(f8).rearrange(
                                "p b c (d two) -> p b two (c d)", two=2
                            )
                        for ko in range(KO_BF):
                            nc.tensor.matmul(
                                po[:, :pw],
                                xt_bf[:, ko, :],
                                wtg_bf[g][:, :, ko, :],
                                start=(ko == 0),
                                stop=False,
                            )
                        for b in range(NB8):
                            nc.tensor.matmul(
                                po[:, :pw],
                                xt8[:, b, :, :],
                                w8v[:, b, :, :]
                                if W_STRIDED_MOVING
                                else wt8[g][:, b, :, :],
                                start=False,
                                stop=(b == NB8 - 1),
                                perf_mode=DR,
                            )
                        # out = psum * alpha (ACT), then += bias (GpSimd)
                        osb = op.tile([P, 512], f32, tag="osb", name=f"osb_{ms}_{g}")
                        nc.scalar.activation(
                            osb[:, :width],
                            po[:, :width],
                            Act.Copy,
                            scale=float(alpha_f),
                        )
                        nc.gpsimd.tensor_add(
                            osb[:, :width],
                            osb[:, :width],
                            bias_sb[:, n0 : n0 + width],
                        )
                        # ACT HWDGE ring: avoids head-of-line blocking of the
                        # next ms's loads on the sync ring
                        nc.scalar.dma_start(
                            o_d[ms * P : (ms + 1) * P, n0 : n0 + width],
                            osb[:, :width],
                        )

                for ms in range(M_SUBS + 1):
                    if ms < M_SUBS:
                        emit_prep(ms)
                    if ms >= 1:
                        emit_compute(ms - 1)

    nc.compile()
    return nc


_CACHE = {}


def _get_nc(alpha_f):
    key = float(alpha_f)
    if key not in _CACHE:
        _CACHE[key] = _build(key)
    return _CACHE[key]


def kernel(x, w, alpha, bias):
    from concourse.bass_utils import run_bass_kernel_spmd

    alpha2 = np.ascontiguousarray(np.asarray(alpha, dtype=np.float32).reshape(1))
    nc = _get_nc(alpha2[0])
    x2 = np.ascontiguousarray(np.asarray(x, dtype=np.float32).reshape(TOK, DIN))
    in_maps = []
    for c in range(NCORES):
        in_maps.append(
            {
                "x": x2,
                "w": np.ascontiguousarray(w[c * DOUT_SH : (c + 1) * DOUT_SH]),
                "alpha": alpha2,
                "bias": np.ascontiguousarray(bias[c * DOUT_SH : (c + 1) * DOUT_SH]),
            }
        )
    res = run_bass_kernel_spmd(nc, in_maps, core_ids=list(range(NCORES)))
    outs = [res.results[c]["out"] for c in range(NCORES)]
    out = np.concatenate(outs, axis=1).reshape(B, S, DOUT)
    return np.ascontiguousarray(out.astype(np.float32))
